# revision 17
# baseline (speedup 1.0000x reference)
"""Trainium2 Bass kernel for a dense transformer block (B=4, T=2048, C=1024, H=16).

Sharding: data-parallel over tokens. Core i owns batch b=i//2, token-half i%2
(1024 tokens). Each core redundantly computes LN1/K/V for its batch's full 2048
tokens (+12% PE) so there are no collectives at all.

Everything on-chip is feature-major ([C, tok]); the host pre-transposes x and
post-transposes the output, so the kernel needs no on-chip transposes:
  - LN stats via ones-matrix matmuls on PE (partition reduction); LN2 stats are
    fused into the attention-proj epilogue so the MLP can start immediately,
  - QKV/proj/MLP matmuls contract C on the partition axis,
  - attention scores computed as scores^T [keys, queries] so softmax's exp is a
    single ACT pass per multi-bank PSUM tile and the denominator rides the AV
    matmul as a 65th all-ones column of V,
  - per-head q is stored twice with the other head's rows zeroed, so score
    matmuls contract K=128 (mixing K=64 matmuls at partition bases 0 and 64 in
    one program faults the runtime).
Matmuls run in bf16 (f32 PSUM accumulation); the residual stream stays f32.
LN scale (g) is folded into weight rows and LN shift (beta) into biases on the
host, so on-chip LN is a pure normalize.

PSUM layout: sc4 [128,4,512] (4 banks) + sc2 [128,2,512] (2 banks) + 2x mm
[128,512]. Attention alternates 4-tile and 2-tile exp batches between sc4 and
sc2 so ACT (the attention bottleneck) pays fewer per-call overheads; QKV/MLP
use all 8 bank-views as independent accumulators.
"""

import sys

if "/opt/trn_rl_repo" not in sys.path:
    sys.path.insert(0, "/opt/trn_rl_repo")

import numpy as np
import ml_dtypes

B, T, C, H, HD = 4, 2048, 1024, 16, 64
FF = 4 * C
TO = T // 2          # tokens owned per core
NCC = C // 128       # 8
NFC = FF // 128      # 32
EPS = 1e-5
SCALE = C ** -0.5    # 1/32
BF16 = ml_dtypes.bfloat16
F8NP = ml_dtypes.float8_e4m3

_BUILT = None


def _emit(nc, tc, aps, has_bv, has_bp):
    from concourse import mybir
    from concourse.bass import ts
    F32 = mybir.dt.float32
    BF = mybir.dt.bfloat16
    F8 = mybir.dt.float8e4
    AF = mybir.ActivationFunctionType
    ADD = mybir.AluOpType.add
    MUL = mybir.AluOpType.mult
    MAX = mybir.AluOpType.max
    DR = mybir.MatmulPerfMode.DoubleRow
    S32 = 1.0 / 32.0
    from contextlib import ExitStack

    xT, wq, wk, wv, wproj, w1, w2, x2d, outT = (
        aps["xT"], aps["wq"], aps["wk"], aps["wv"], aps["wproj"], aps["w1"],
        aps["w2"], aps["x2d"], aps["outT"])
    vda = aps["vda"]

    ctx = ExitStack()
    with ctx:
        const = ctx.enter_context(tc.tile_pool(name="const", bufs=1))
        misc = ctx.enter_context(tc.tile_pool(name="misc", bufs=2))
        wpool = ctx.enter_context(tc.tile_pool(name="wpool", bufs=8))
        psum = ctx.enter_context(tc.tile_pool(name="psum", bufs=1, space="PSUM"))

        def ps_sc2():
            return psum.tile([128, 2, 512], F32, name="ps_sc2", tag="sc2", bufs=2)

        def ps_stp():
            return psum.tile([128, 2, 512], F32, name="ps_stp", tag="stp", bufs=1)

        def ps_mm():
            return psum.tile([128, 512], F32, name="ps_mm", tag="mm", bufs=2)

        def _rot_gen():
            while True:
                t = ps_sc2()
                yield t[:, 0, :]
                yield t[:, 1, :]
                yield ps_mm()
                yield ps_mm()

        _rot = _rot_gen()

        def ps_rot():
            return next(_rot)

        def acc8():
            """8 independent [128,512] psum accumulator views (all 8 banks)."""
            t = [ps_sc2(), ps_sc2(), ps_stp()]
            return ([x[:, j, :] for x in t for j in range(2)]
                    + [ps_mm(), ps_mm()])

        # constants / biases
        ones_sc = const.tile([128, 128], BF, name="ones_sc")
        nc.vector.memset(ones_sc, 1.0 / C)
        eps_sb = const.tile([128, 1], F32, name="eps_sb")
        nc.vector.memset(eps_sb, EPS)
        bq_sb = const.tile([128, 8], F32, name="bq_sb")
        bk_sb = const.tile([128, 8], F32, name="bk_sb")
        bp_sb = const.tile([128, 8], F32, name="bp_sb")
        b2_sb = const.tile([128, 8], F32, name="b2_sb")
        b1_sb = const.tile([128, 32], F32, name="b1_sb")
        nc.sync.dma_start(out=bq_sb, in_=aps["bq"])
        nc.sync.dma_start(out=bk_sb, in_=aps["bk"])
        nc.sync.dma_start(out=bp_sb, in_=aps["bp"])
        nc.sync.dma_start(out=b2_sb, in_=aps["b2c"])
        nc.sync.dma_start(out=b1_sb, in_=aps["b1c"])
        if has_bv:
            bv_sb = const.tile([1, 1024], F32, name="bv_sb")
            nc.sync.dma_start(out=bv_sb, in_=aps["bvrow"])
            bvb = const.tile([128, 1024], F32, name="bvb")
            nc.gpsimd.partition_broadcast(bvb, bv_sb)

        def stats_finish(st, tag, bufs=4):
            """st: psum [:,0]=mean, [:,1]=E[x^2] (rows replicated).
            Returns (mu_sb, s_sb) sbuf tiles (tagged so lifetime fits usage)."""
            mu_sb = misc.tile([128, 512], F32, name="mu_sb", tag=tag, bufs=bufs)
            nc.scalar.copy(out=mu_sb, in_=st[:, 0, :])
            musq = misc.tile([128, 512], F32, name="musq", tag="stat", bufs=2)
            nc.scalar.square(out=musq, in_=st[:, 0, :])
            var = misc.tile([128, 512], F32, name="var", tag="stat", bufs=2)
            nc.vector.tensor_sub(out=var, in0=st[:, 1, :], in1=musq)
            sd = misc.tile([128, 512], F32, name="sd", tag="stat", bufs=2)
            nc.scalar.activation(out=sd, in_=var, func=AF.Sqrt, bias=eps_sb)
            s_sb = misc.tile([128, 512], F32, name="s_sb", tag=tag, bufs=bufs)
            nc.vector.reciprocal(out=s_sb, in_=sd)
            return mu_sb, s_sb

        def ln_stats(lnt, src_dram, tcg):
            """Stats pass for one 512-token chunk: DMA loads, bf16 casts,
            PE stats matmuls + finish chain. Returns (xk, mu, s)."""
            st = ps_stp()
            xk = []
            for c in range(NCC):
                xs = lnt.tile([128, 512], F32, name="xs", tag="xs", bufs=4)
                nc.sync.dma_start(out=xs, in_=src_dram[c, :, ts(tcg, 512)])
                xbf = lnt.tile([128, 512], BF, name="xbf", tag="xk", bufs=16)
                nc.scalar.copy(out=xbf, in_=xs)
                xk.append(xbf)
                xsq = lnt.tile([128, 512], BF, name="xsq", tag="xsq", bufs=3)
                nc.vector.tensor_mul(out=xsq, in0=xbf, in1=xbf)
                nc.tensor.matmul(st[:, 0, :], ones_sc, xbf,
                                 start=(c == 0), stop=(c == NCC - 1),
                                 skip_group_check=True)
                nc.tensor.matmul(st[:, 1, :], ones_sc, xsq,
                                 start=(c == 0), stop=(c == NCC - 1),
                                 skip_group_check=True)
            mu_sb, s_sb = stats_finish(st, "mstat", bufs=3)
            return xk, mu_sb, s_sb

        def ln_apply(lnt, stats, h_dst):
            xk, mu_sb, s_sb = stats
            for c in range(NCC):
                d = lnt.tile([128, 512], F32, name="d", tag="xs", bufs=4)
                nc.vector.tensor_sub(out=d, in0=xk[c], in1=mu_sb)
                nc.vector.tensor_mul(out=h_dst[:, c, :], in0=d, in1=s_sb)

        # ============ scope A: LN1 + QKV + attention + proj ============
        ln2_stats = {}
        with tc.tile_pool(name="attn", bufs=1) as attn:
            k_sb = attn.tile([128, 8, T], F8, name="k_sb")
            qz0 = attn.tile([128, 8, TO], F8, name="qz0")
            qz1 = attn.tile([128, 8, TO], F8, name="qz1")
            o_sb = attn.tile([128, 8, TO], F8, name="o_sb")
            nc.vector.memset(qz0[64:128, :, :], 0.0)
            nc.vector.memset(qz1[0:64, :, :], 0.0)

            # ---- LN1 software-pipelined one chunk ahead of K/V/Q matmuls so
            #      the PE never waits on a stats chain; all weights prefetched
            #      up front. V is staged through DRAM so everything fits. ----
            with tc.tile_pool(name="lnp", bufs=2) as lnp, \
                 tc.tile_pool(name="vst", bufs=2) as vst, \
                 tc.tile_pool(name="wkv", bufs=8) as wkv, \
                 tc.tile_pool(name="lnt", bufs=4) as lnt:
                # first chunk's x loads go ahead of the (bigger) weight
                # prefetch so the PE can start stats immediately
                stats_next = ln_stats(lnt, xT, 0)
                wt = [wkv.tile([128, 2, 1024], F8, name="wt", tag="wkvt")
                      for _ in range(4)]
                for p in range(4):
                    nc.sync.dma_start(out=wt[p], in_=wk[p])
                wtv = [wkv.tile([128, 2, 1024], F8, name="wtv", tag="wkvt")
                       for _ in range(4)]
                for p in range(4):
                    nc.sync.dma_start(out=wtv[p], in_=wv[p])
                wtq = [wpool.tile([128, 2, 1024], F8, name="wtq", tag="w")
                       for _ in range(4)]
                for p in range(4):
                    nc.sync.dma_start(out=wtq[p], in_=wq[p])
                for tcg in range(4):
                    stats_cur = stats_next
                    if tcg < 3:
                        stats_next = ln_stats(lnt, xT, tcg + 1)
                    h_t = lnp.tile([128, 8, 512], F8, name="h_t", tag="h")
                    ln_apply(lnt, stats_cur, h_t)
                    for m in range(NCC):
                        kp = ps_rot()
                        for p in range(4):
                            nc.tensor.matmul(kp, wt[p][:, :, ts(m, 128)],
                                             h_t[:, 2 * p:2 * p + 2, :],
                                             start=(p == 0), stop=(p == 3),
                                             perf_mode=DR,
                                             skip_group_check=True)
                        nc.scalar.activation(
                            out=k_sb[:, m, ts(tcg, 512)], in_=kp,
                            func=AF.Identity, bias=bk_sb[:, m:m + 1],
                            scale=S32)
                    for nch in range(2):
                        for tt in range(4):
                            vp = ps_rot()
                            for p in range(4):
                                nc.tensor.matmul(vp,
                                                 h_t[:, 2 * p:2 * p + 2,
                                                     ts(tt, 128)],
                                                 wtv[p][:, :, ts(nch, 512)],
                                                 start=(p == 0), stop=(p == 3),
                                                 perf_mode=DR,
                                                 skip_group_check=True)
                            vt = vst.tile([128, 8, 65], F8, name="vt", tag="vt")
                            nc.vector.memset(vt[:, :, 64:65], 1.0)
                            src = vp.rearrange("p (h d) -> p h d", h=8)
                            if has_bv:
                                bslice = bvb[:, ts(nch, 512)].rearrange(
                                    "p (h d) -> p h d", h=8)
                                nc.vector.scalar_tensor_tensor(
                                    out=vt[:, :, 0:64], in0=src, scalar=S32,
                                    in1=bslice, op0=MUL, op1=ADD)
                            else:
                                nc.vector.tensor_scalar_mul(
                                    out=vt[:, :, 0:64], in0=src, scalar1=S32)
                            nc.sync.dma_start(
                                out=vda[tcg * 4 + tt, :,
                                        nch * 8:(nch + 1) * 8, :],
                                in_=vt)
                    if tcg < 2:
                        # Q projection for own tokens, inside the loop so PE
                        # work covers the next chunk's stats chain
                        for m in range(NCC):
                            qp = ps_rot()
                            for p in range(4):
                                nc.tensor.matmul(qp, wtq[p][:, :, ts(m, 128)],
                                                 h_t[:, 2 * p:2 * p + 2, :],
                                                 start=(p == 0), stop=(p == 3),
                                                 perf_mode=DR,
                                                 skip_group_check=True)
                            nc.scalar.activation(
                                out=qz0[0:64, m, ts(tcg, 512)], in_=qp[0:64, :],
                                func=AF.Identity, bias=bq_sb[0:64, m:m + 1],
                                scale=S32)
                            nc.scalar.activation(
                                out=qz1[64:128, m, ts(tcg, 512)],
                                in_=qp[64:128, :],
                                func=AF.Identity, bias=bq_sb[64:128, m:m + 1],
                                scale=S32)

            # ---- attention + proj + MLP, phase-interleaved:
            #   B: attention qc0 (ACT exp-bound)
            #   C: attention qc1 with proj0/LN2(0)/MLP1(0) matmuls filling PE
            #   D: proj1 + MLP2(0) + LN2(1) + MLP(1)  (PE-dense)
            # x2 residual stays in SBUF as bf16 tiles (no DRAM round trip).
            with tc.tile_pool(name="vap", bufs=1) as vap, \
                 tc.tile_pool(name="expp", bufs=3) as expp, \
                 tc.tile_pool(name="stage", bufs=2) as stage, \
                 tc.tile_pool(name="x2p", bufs=16) as x2p, \
                 tc.tile_pool(name="mlp", bufs=1) as mlp, \
                 tc.tile_pool(name="h2p", bufs=2) as h2p, \
                 tc.tile_pool(name="outp", bufs=3) as outp:
                v_aug = vap.tile([128, 16, 16, 65], F8, name="v_aug")
                for ttg in range(16):
                    for nch in range(2):
                        nc.sync.dma_start(
                            out=v_aug[:, ttg, nch * 8:(nch + 1) * 8, :],
                            in_=vda[ttg, :, nch * 8:(nch + 1) * 8, :])

                def attn_head(qc, h, stp_ok):
                    hp = h // 2
                    p0 = (h % 2) * 64
                    qz = qz0 if h % 2 == 0 else qz1
                    avp = ps_mm()
                    for g in range(8):
                        scp = ps_stp() if (stp_ok and g % 3 == 2) else ps_sc2()
                        for j in range(2):
                            sk = g * 2 + j
                            nc.tensor.matmul(scp[:, j, :],
                                             k_sb[:, hp, ts(sk, 128)],
                                             qz[:, hp, ts(qc, 512)],
                                             start=True, stop=True)
                        et = expp.tile([128, 2, 512], F8, name="et", tag="et")
                        nc.scalar.activation(out=et, in_=scp, func=AF.Exp,
                                             scale=SCALE)
                        nc.tensor.matmul(avp[0:65, :],
                                         v_aug[:, 2 * g:2 * g + 2, h, :], et,
                                         start=(g == 0), stop=(g == 7),
                                         perf_mode=DR, skip_group_check=True)
                    r_t = misc.tile([1, 512], F32, name="r_t", tag="r", bufs=2)
                    nc.vector.reciprocal(out=r_t, in_=avp[64:65, :])
                    rb_t = misc.tile([64, 512], F32, name="rb_t", tag="rb",
                                     bufs=2)
                    nc.gpsimd.partition_broadcast(rb_t, r_t)
                    nc.vector.scalar_tensor_tensor(
                        out=o_sb[p0:p0 + 64, hp, ts(qc, 512)],
                        in0=avp[0:64, :], scalar=32.0, in1=rb_t,
                        op0=MUL, op1=MUL)

                x2t = {0: [], 1: []}

                def proj_qc(qc, wtp):
                    st2 = ps_stp()
                    for m in range(NCC):
                        pp = ps_rot()
                        for p in range(4):
                            nc.tensor.matmul(pp, wtp[p][:, :, ts(m, 128)],
                                             o_sb[:, 2 * p:2 * p + 2,
                                                  ts(qc, 512)],
                                             start=(p == 0), stop=(p == 3),
                                             perf_mode=DR,
                                             skip_group_check=True)
                        xres = stage.tile([128, 512], F32, name="xres",
                                          tag="xres")
                        nc.sync.dma_start(out=xres, in_=xT[m, :, ts(qc, 512)])
                        xt = x2p.tile([128, 512], BF, name="x2t", tag="x2")
                        if has_bp:
                            ptmp = stage.tile([128, 512], F32, name="ptmp",
                                              tag="ptmp")
                            nc.scalar.activation(out=ptmp, in_=pp,
                                                 func=AF.Identity,
                                                 bias=bp_sb[:, m:m + 1],
                                                 scale=2.0 ** -10)
                            nc.vector.tensor_add(out=xt, in0=ptmp, in1=xres)
                        else:
                            nc.vector.scalar_tensor_tensor(
                                out=xt, in0=pp, scalar=2.0 ** -10,
                                in1=xres, op0=MUL, op1=ADD)
                        x2t[qc].append(xt)
                        xq2 = stage.tile([128, 512], BF, name="xq2",
                                         tag="xq2", bufs=3)
                        nc.vector.tensor_mul(out=xq2, in0=xt, in1=xt)
                        nc.tensor.matmul(st2[:, 0, :], ones_sc, xt,
                                         start=(m == 0), stop=(m == NCC - 1),
                                         skip_group_check=True)
                        nc.tensor.matmul(st2[:, 1, :], ones_sc, xq2,
                                         start=(m == 0), stop=(m == NCC - 1),
                                         skip_group_check=True)
                    return stats_finish(st2, "mstat", bufs=3)

                def ln2_apply(qc, mu2, s2):
                    h2 = h2p.tile([128, 8, 512], BF, name="h2", tag="h2")
                    for c in range(NCC):
                        d2 = stage.tile([128, 512], F32, name="d2", tag="d2",
                                        bufs=4)
                        nc.vector.tensor_sub(out=d2, in0=x2t[qc][c], in1=mu2)
                        nc.vector.tensor_mul(out=h2[:, c, :], in0=d2, in1=s2)
                    return h2

                def mlp1_c(h2, z_sb, mg):
                    """One 512-wide W1 column group as two 2-chunk halves on
                    the stp psum pair; z epilogue on DVE so the exp stream on
                    ACT is undisturbed."""
                    w1t = [wpool.tile([128, 512], BF, name="w1t", tag="w1s",
                                      bufs=12) for _ in range(NCC)]
                    for k in range(NCC):
                        nc.sync.dma_start(out=w1t[k], in_=w1[mg, k])
                    for half in range(2):
                        acc = ps_stp()
                        for k in range(NCC):
                            for m2 in range(2):
                                nc.tensor.matmul(
                                    acc[:, m2, :],
                                    w1t[k][:, ts(half * 2 + m2, 128)],
                                    h2[:, k, :], start=(k == 0),
                                    stop=(k == NCC - 1),
                                    skip_group_check=True)
                        for m2 in range(2):
                            m = mg * 4 + half * 2 + m2
                            nc.vector.tensor_scalar(
                                out=z_sb[:, m, :], in0=acc[:, m2, :],
                                scalar1=b1_sb[:, m:m + 1], scalar2=0.0,
                                op0=ADD, op1=MAX)

                def mlp1_d(h2, z_sb):
                    for mg in range(8):
                        if mg % 2 == 0:
                            t_a, t_b = ps_sc2(), ps_sc2()
                            accs = [t_a[:, 0, :], t_a[:, 1, :],
                                    t_b[:, 0, :], t_b[:, 1, :]]
                        else:
                            t_a = ps_stp()
                            accs = [t_a[:, 0, :], t_a[:, 1, :],
                                    ps_mm(), ps_mm()]
                        w1t = [wpool.tile([128, 512], BF, name="w1t",
                                          tag="w1s", bufs=12)
                               for _ in range(NCC)]
                        for k in range(NCC):
                            nc.sync.dma_start(out=w1t[k], in_=w1[mg, k])
                        for k in range(NCC):
                            for m4 in range(4):
                                nc.tensor.matmul(accs[m4],
                                                 w1t[k][:, ts(m4, 128)],
                                                 h2[:, k, :], start=(k == 0),
                                                 stop=(k == NCC - 1),
                                                 skip_group_check=True)
                        for m4 in range(4):
                            m = mg * 4 + m4
                            nc.vector.tensor_scalar(
                                out=z_sb[:, m, :], in0=accs[m4],
                                scalar1=b1_sb[:, m:m + 1], scalar2=0.0,
                                op0=ADD, op1=MAX)

                def mlp2(qc, z_sb):
                    accs = acc8()
                    for k in range(NFC):
                        w2t = wpool.tile([128, 1024], BF, name="w2t",
                                         tag="w2s", bufs=3)
                        nc.sync.dma_start(out=w2t, in_=w2[k])
                        for m in range(8):
                            nc.tensor.matmul(accs[m], w2t[:, ts(m, 128)],
                                             z_sb[:, k, :], start=(k == 0),
                                             stop=(k == NFC - 1),
                                             skip_group_check=True)
                    for m in range(8):
                        ot = outp.tile([128, 512], F32, name="ot", tag="ot")
                        nc.vector.scalar_tensor_tensor(
                            out=ot, in0=accs[m], scalar=b2_sb[:, m:m + 1],
                            in1=x2t[qc][m], op0=ADD, op1=ADD)
                        nc.sync.dma_start(out=outT[m, :, ts(qc, 512)], in_=ot)

                wtp0 = [wpool.tile([128, 2, 1024], F8, name="wtp", tag="w")
                        for _ in range(4)]
                for p in range(4):
                    nc.sync.dma_start(out=wtp0[p], in_=wproj[p])
                # phase B: attention qc0 (ACT exp-bound; PE has slack)
                for h in range(H):
                    attn_head(0, h, stp_ok=True)
                # phase C preamble: proj0 -> x2(qc0) in SBUF, LN2(qc0)
                mu20, s20 = proj_qc(0, wtp0)
                h2_0 = ln2_apply(0, mu20, s20)
                z_0 = mlp.tile([128, 32, 512], BF, name="z_sb", tag="z")
                wtp1 = [wpool.tile([128, 2, 1024], F8, name="wtp", tag="w")
                        for _ in range(4)]
                for p in range(4):
                    nc.sync.dma_start(out=wtp1[p], in_=wproj[p])
                # phase C: attention qc1, MLP1(qc0) column groups interleaved
                # between heads to fill the PE while ACT streams exps
                for h in range(H):
                    attn_head(1, h, stp_ok=False)
                    if h >= 2 and h % 2 == 0:
                        mlp1_c(h2_0, z_0, (h - 2) // 2)
                mlp1_c(h2_0, z_0, 7)
                # phase D: PE-dense tail
                mu21, s21 = proj_qc(1, wtp1)
                mlp2(0, z_0)
                h2_1 = ln2_apply(1, mu21, s21)
                z_1 = mlp.tile([128, 32, 512], BF, name="z_sb", tag="z")
                mlp1_d(h2_1, z_1)
                mlp2(1, z_1)


def _build(has_bv, has_bp):
    from concourse import bacc, mybir, tile
    F32 = mybir.dt.float32
    BF = mybir.dt.bfloat16
    F8 = mybir.dt.float8e4

    nc = bacc.Bacc("TRN2", target_bir_lowering=False, debug=False,
                   enable_asserts=False, num_devices=8)
    aps = {}
    aps["xT"] = nc.dram_tensor("xT", [8, 128, T], F32, kind="ExternalInput").ap()
    for n in ("wq", "wk", "wv", "wproj"):
        aps[n] = nc.dram_tensor(n, [4, 128, 2, 1024], F8,
                                kind="ExternalInput").ap()
    aps["w1"] = nc.dram_tensor("w1", [8, 8, 128, 512], BF, kind="ExternalInput").ap()
    aps["w2"] = nc.dram_tensor("w2", [32, 128, 1024], BF, kind="ExternalInput").ap()
    for n in ("bq", "bk", "bp", "b2c"):
        aps[n] = nc.dram_tensor(n, [128, 8], F32, kind="ExternalInput").ap()
    aps["b1c"] = nc.dram_tensor("b1c", [128, 32], F32, kind="ExternalInput").ap()
    if has_bv:
        aps["bvrow"] = nc.dram_tensor("bvrow", [1, 1024], F32,
                                      kind="ExternalInput").ap()
    aps["x2d"] = nc.dram_tensor("x2d", [8, 128, TO], F32).ap()
    aps["vda"] = nc.dram_tensor("vda", [16, 128, 16, 65], F8).ap()
    aps["outT"] = nc.dram_tensor("outT", [8, 128, TO], F32,
                                 kind="ExternalOutput").ap()

    with tile.TileContext(nc) as tcx:
        _emit(nc, tcx, aps, has_bv, has_bp)
    nc.compile()
    return nc


def _prep_inputs(x, Wq, Wk, Wv, Wproj, bproj, W1, b1, W2, b2, g1, be1, g2, be2):
    """Host-side prep: fold LN affine into weights/biases, cast, lay out."""
    x = np.asarray(x, np.float32)
    g1 = np.asarray(g1, np.float32)
    be1 = np.asarray(be1, np.float32)
    g2 = np.asarray(g2, np.float32)
    be2 = np.asarray(be2, np.float32)

    def to2d(w):  # (H, C, hd) -> (C, H*hd)
        return np.asarray(w, np.float32).transpose(1, 0, 2).reshape(C, C)

    wq2, wk2, wv2 = to2d(Wq), to2d(Wk), to2d(Wv)
    Wproj = np.asarray(Wproj, np.float32)
    W1 = np.asarray(W1, np.float32)
    W2 = np.asarray(W2, np.float32)

    wq_e, wk_e, wv_e = g1[:, None] * wq2, g1[:, None] * wk2, g1[:, None] * wv2
    w1_e = g2[:, None] * W1
    bias_q = be1 @ wq2
    bias_k = be1 @ wk2
    bias_v = be1 @ wv2
    bias_1 = np.asarray(b1, np.float32) + be2 @ W1

    def wpair8(w):
        """(C, N) f32, scaled x32 -> [4, 128, 2, N] fp8e4m3 DoubleRow pairs:
        pair p slot j holds contraction rows 256p+128j .. +127."""
        w8 = np.clip(w * 32.0, -240.0, 240.0).astype(F8NP)
        return np.ascontiguousarray(
            w8.reshape(4, 2, 128, -1).transpose(0, 2, 1, 3))

    def bvec(v):  # (N,) -> (128, N//128) partition-major
        return np.ascontiguousarray(np.asarray(v, np.float32).reshape(-1, 128).T)

    shared = {
        "wq": wpair8(wq_e), "wk": wpair8(wk_e), "wv": wpair8(wv_e),
        "wproj": wpair8(Wproj),
        "w1": np.ascontiguousarray(
            w1_e.reshape(NCC, 128, 8, 512).transpose(2, 0, 1, 3).astype(BF16)),
        "w2": np.ascontiguousarray(W2.reshape(NFC, 128, C).astype(BF16)),
        "bq": bvec(bias_q), "bk": bvec(bias_k),
        "bp": bvec(np.asarray(bproj, np.float32)),
        "b2c": bvec(np.asarray(b2, np.float32)), "b1c": bvec(bias_1),
    }
    has_bv = bool(np.any(bias_v != 0.0))
    has_bp = bool(np.any(np.asarray(bproj, np.float32) != 0.0))
    if has_bv:
        shared["bvrow"] = np.ascontiguousarray(bias_v.reshape(1, C))

    in_maps = []
    for core in range(8):
        b, half = core // 2, core % 2
        xt = x[b].T  # (C, T)
        own = xt[:, half * TO:(half + 1) * TO]
        oth = xt[:, (1 - half) * TO:(2 - half) * TO]
        m = dict(shared)
        m["xT"] = np.ascontiguousarray(
            np.concatenate([own, oth], axis=1).reshape(NCC, 128, T))
        in_maps.append(m)
    return in_maps, has_bv, has_bp


def kernel(x, Wq, Wk, Wv, Wproj, bproj, W1, b1, W2, b2, g1, be1, g2, be2):
    global _BUILT
    from concourse.bass_utils import run_bass_kernel_spmd

    in_maps, has_bv, has_bp = _prep_inputs(x, Wq, Wk, Wv, Wproj, bproj, W1, b1,
                                           W2, b2, g1, be1, g2, be2)
    if _BUILT is None or _BUILT[1] != (has_bv, has_bp):
        _BUILT = (_build(has_bv, has_bp), (has_bv, has_bp))
    nc = _BUILT[0]
    res = run_bass_kernel_spmd(nc, in_maps, core_ids=list(range(8)))
    out = np.empty((B, T, C), np.float32)
    for core in range(8):
        b, half = core // 2, core % 2
        o = res.results[core]["outT"].reshape(C, TO)  # (feature, token)
        out[b, half * TO:(half + 1) * TO, :] = o.T
    return out



# revision 20
# speedup vs baseline: 1.0488x; 1.0488x over previous
"""Trainium2 Bass kernel for a dense transformer block (B=4, T=2048, C=1024, H=16).

Sharding: data-parallel over tokens. Core i owns batch b=i//2, token-half i%2
(1024 tokens). Each core redundantly computes LN1/K/V for its batch's full 2048
tokens (+12% PE) so there are no collectives at all.

Everything on-chip is feature-major ([C, tok]); the host pre-transposes x and
post-transposes the output, so the kernel needs no on-chip transposes:
  - LN stats via ones-matrix matmuls on PE (partition reduction); LN2 stats are
    fused into the attention-proj epilogue so the MLP can start immediately,
  - QKV/proj/MLP matmuls contract C on the partition axis,
  - attention scores computed as scores^T [keys, queries] so softmax's exp is a
    single ACT pass per multi-bank PSUM tile and the denominator rides the AV
    matmul as a 65th all-ones column of V,
  - per-head q is stored twice with the other head's rows zeroed, so score
    matmuls contract K=128 (mixing K=64 matmuls at partition bases 0 and 64 in
    one program faults the runtime).
Matmuls run in bf16 (f32 PSUM accumulation); the residual stream stays f32.
LN scale (g) is folded into weight rows and LN shift (beta) into biases on the
host, so on-chip LN is a pure normalize.

PSUM layout: sc4 [128,4,512] (4 banks) + sc2 [128,2,512] (2 banks) + 2x mm
[128,512]. Attention alternates 4-tile and 2-tile exp batches between sc4 and
sc2 so ACT (the attention bottleneck) pays fewer per-call overheads; QKV/MLP
use all 8 bank-views as independent accumulators.
"""

import sys

if "/opt/trn_rl_repo" not in sys.path:
    sys.path.insert(0, "/opt/trn_rl_repo")

import numpy as np
import ml_dtypes

B, T, C, H, HD = 4, 2048, 1024, 16, 64
FF = 4 * C
TO = T // 2          # tokens owned per core
NCC = C // 128       # 8
NFC = FF // 128      # 32
EPS = 1e-5
SCALE = C ** -0.5    # 1/32
BF16 = ml_dtypes.bfloat16
F8NP = ml_dtypes.float8_e4m3

_BUILT = None


def _emit(nc, tc, aps, has_bv, has_bp):
    from concourse import mybir
    from concourse.bass import ts
    F32 = mybir.dt.float32
    BF = mybir.dt.bfloat16
    F8 = mybir.dt.float8e4
    AF = mybir.ActivationFunctionType
    ADD = mybir.AluOpType.add
    MUL = mybir.AluOpType.mult
    MAX = mybir.AluOpType.max
    DR = mybir.MatmulPerfMode.DoubleRow
    S32 = 1.0 / 32.0
    from contextlib import ExitStack

    xT, wq, wk, wv, wproj, w1, w2, x2d, outT = (
        aps["xT"], aps["wq"], aps["wk"], aps["wv"], aps["wproj"], aps["w1"],
        aps["w2"], aps["x2d"], aps["outT"])
    vda = aps["vda"]

    ctx = ExitStack()
    with ctx:
        const = ctx.enter_context(tc.tile_pool(name="const", bufs=1))
        misc = ctx.enter_context(tc.tile_pool(name="misc", bufs=2))
        wpool = ctx.enter_context(tc.tile_pool(name="wpool", bufs=8))
        psum = ctx.enter_context(tc.tile_pool(name="psum", bufs=1, space="PSUM"))

        def ps_sc2():
            return psum.tile([128, 2, 512], F32, name="ps_sc2", tag="sc2", bufs=2)

        def ps_stp():
            return psum.tile([128, 2, 512], F32, name="ps_stp", tag="stp", bufs=1)

        def ps_mm():
            return psum.tile([128, 512], F32, name="ps_mm", tag="mm", bufs=2)

        def _rot_gen():
            while True:
                t = ps_sc2()
                yield t[:, 0, :]
                yield t[:, 1, :]
                yield ps_mm()
                yield ps_mm()

        _rot = _rot_gen()

        def ps_rot():
            return next(_rot)

        def acc8():
            """8 independent [128,512] psum accumulator views (all 8 banks)."""
            t = [ps_sc2(), ps_sc2(), ps_stp()]
            return ([x[:, j, :] for x in t for j in range(2)]
                    + [ps_mm(), ps_mm()])

        # constants / biases
        ones_sc = const.tile([128, 128], BF, name="ones_sc")
        nc.vector.memset(ones_sc, 1.0 / C)
        eps_sb = const.tile([128, 1], F32, name="eps_sb")
        nc.vector.memset(eps_sb, EPS)
        bq_sb = const.tile([128, 8], F32, name="bq_sb")
        bk_sb = const.tile([128, 8], F32, name="bk_sb")
        bp_sb = const.tile([128, 8], F32, name="bp_sb")
        b2_sb = const.tile([128, 8], F32, name="b2_sb")
        b1_sb = const.tile([128, 32], F32, name="b1_sb")
        nc.sync.dma_start(out=bq_sb, in_=aps["bq"])
        nc.sync.dma_start(out=bk_sb, in_=aps["bk"])
        nc.sync.dma_start(out=bp_sb, in_=aps["bp"])
        nc.sync.dma_start(out=b2_sb, in_=aps["b2c"])
        nc.sync.dma_start(out=b1_sb, in_=aps["b1c"])
        if has_bv:
            bv_sb = const.tile([1, 1024], F32, name="bv_sb")
            nc.sync.dma_start(out=bv_sb, in_=aps["bvrow"])
            bvb = const.tile([128, 1024], F32, name="bvb")
            nc.gpsimd.partition_broadcast(bvb, bv_sb)

        def stats_finish(st, tag, bufs=4):
            """st: psum [:,0]=mean, [:,1]=E[x^2] (rows replicated).
            Returns (mu_sb, s_sb) sbuf tiles (tagged so lifetime fits usage)."""
            mu_sb = misc.tile([128, 512], F32, name="mu_sb", tag=tag, bufs=bufs)
            nc.scalar.copy(out=mu_sb, in_=st[:, 0, :])
            musq = misc.tile([128, 512], F32, name="musq", tag="stat", bufs=2)
            nc.scalar.square(out=musq, in_=st[:, 0, :])
            var = misc.tile([128, 512], F32, name="var", tag="stat", bufs=2)
            nc.vector.tensor_sub(out=var, in0=st[:, 1, :], in1=musq)
            sd = misc.tile([128, 512], F32, name="sd", tag="stat", bufs=2)
            nc.scalar.activation(out=sd, in_=var, func=AF.Sqrt, bias=eps_sb)
            s_sb = misc.tile([128, 512], F32, name="s_sb", tag=tag, bufs=bufs)
            nc.vector.reciprocal(out=s_sb, in_=sd)
            return mu_sb, s_sb

        def ln_stats(lnt, src_dram, tcg):
            """Stats pass for one 512-token chunk: DMA loads, bf16 casts,
            PE stats matmuls + finish chain. Returns (xk, mu, s)."""
            st = ps_stp()
            xk = []
            for c in range(NCC):
                xs = lnt.tile([128, 512], F32, name="xs", tag="xs", bufs=4)
                nc.sync.dma_start(out=xs, in_=src_dram[c, :, ts(tcg, 512)])
                xbf = lnt.tile([128, 512], BF, name="xbf", tag="xk", bufs=24)
                nc.scalar.copy(out=xbf, in_=xs)
                xk.append(xbf)
                xsq = lnt.tile([128, 512], BF, name="xsq", tag="xsq", bufs=3)
                nc.vector.tensor_mul(out=xsq, in0=xbf, in1=xbf)
                nc.tensor.matmul(st[:, 0, :], ones_sc, xbf,
                                 start=(c == 0), stop=(c == NCC - 1),
                                 skip_group_check=True)
                nc.tensor.matmul(st[:, 1, :], ones_sc, xsq,
                                 start=(c == 0), stop=(c == NCC - 1),
                                 skip_group_check=True)
            mu_sb, s_sb = stats_finish(st, "mstat", bufs=6)
            return xk, mu_sb, s_sb

        def ln_apply(lnt, stats, h_dst):
            xk, mu_sb, s_sb = stats
            for c in range(NCC):
                d = lnt.tile([128, 512], F32, name="d", tag="xs", bufs=4)
                nc.vector.tensor_sub(out=d, in0=xk[c], in1=mu_sb)
                nc.vector.tensor_mul(out=h_dst[:, c, :], in0=d, in1=s_sb)

        # ============ scope A: LN1 + QKV + attention + proj ============
        ln2_stats = {}
        with tc.tile_pool(name="attn", bufs=1) as attn:
            k_sb = attn.tile([128, 8, T], F8, name="k_sb")
            qz0 = attn.tile([128, 8, TO], F8, name="qz0")
            qz1 = attn.tile([128, 8, TO], F8, name="qz1")
            o_sb = attn.tile([128, 8, TO], F8, name="o_sb")
            nc.vector.memset(qz0[64:128, :, :], 0.0)
            nc.vector.memset(qz1[0:64, :, :], 0.0)

            # ---- LN1 software-pipelined one chunk ahead of K/V/Q matmuls so
            #      the PE never waits on a stats chain; all weights prefetched
            #      up front. V is staged through DRAM so everything fits. ----
            with tc.tile_pool(name="lnp", bufs=2) as lnp, \
                 tc.tile_pool(name="vst", bufs=2) as vst, \
                 tc.tile_pool(name="wkv", bufs=8) as wkv, \
                 tc.tile_pool(name="lnt", bufs=4) as lnt:
                # startup order: x chunk 0, K weights, x chunk 1, then the
                # rest — keeps both the stats chain and K matmuls fed early;
                # stats run two chunks ahead of K/V so the finish chains are
                # fully covered by matmul work
                pend = [ln_stats(lnt, xT, 0)]
                wt = [wkv.tile([128, 2, 1024], F8, name="wt", tag="wkvt")
                      for _ in range(4)]
                for p in range(4):
                    nc.sync.dma_start(out=wt[p], in_=wk[p])
                pend.append(ln_stats(lnt, xT, 1))
                wtv = [wkv.tile([128, 2, 1024], F8, name="wtv", tag="wkvt")
                       for _ in range(4)]
                for p in range(4):
                    nc.sync.dma_start(out=wtv[p], in_=wv[p])
                wtq = [wpool.tile([128, 2, 1024], F8, name="wtq", tag="w")
                       for _ in range(4)]
                for p in range(4):
                    nc.sync.dma_start(out=wtq[p], in_=wq[p])
                for tcg in range(4):
                    stats_cur = pend.pop(0)
                    if tcg < 2:
                        pend.append(ln_stats(lnt, xT, tcg + 2))
                    h_t = lnp.tile([128, 8, 512], F8, name="h_t", tag="h")
                    ln_apply(lnt, stats_cur, h_t)
                    for m in range(NCC):
                        kp = ps_rot()
                        for p in range(4):
                            nc.tensor.matmul(kp, wt[p][:, :, ts(m, 128)],
                                             h_t[:, 2 * p:2 * p + 2, :],
                                             start=(p == 0), stop=(p == 3),
                                             perf_mode=DR,
                                             skip_group_check=True)
                        nc.scalar.activation(
                            out=k_sb[:, m, ts(tcg, 512)], in_=kp,
                            func=AF.Identity, bias=bk_sb[:, m:m + 1],
                            scale=S32)
                    for nch in range(2):
                        for tt in range(4):
                            vp = ps_rot()
                            for p in range(4):
                                nc.tensor.matmul(vp,
                                                 h_t[:, 2 * p:2 * p + 2,
                                                     ts(tt, 128)],
                                                 wtv[p][:, :, ts(nch, 512)],
                                                 start=(p == 0), stop=(p == 3),
                                                 perf_mode=DR,
                                                 skip_group_check=True)
                            vt = vst.tile([128, 8, 65], F8, name="vt", tag="vt")
                            nc.vector.memset(vt[:, :, 64:65], 1.0)
                            src = vp.rearrange("p (h d) -> p h d", h=8)
                            if has_bv:
                                bslice = bvb[:, ts(nch, 512)].rearrange(
                                    "p (h d) -> p h d", h=8)
                                nc.vector.scalar_tensor_tensor(
                                    out=vt[:, :, 0:64], in0=src, scalar=S32,
                                    in1=bslice, op0=MUL, op1=ADD)
                            else:
                                nc.vector.tensor_scalar_mul(
                                    out=vt[:, :, 0:64], in0=src, scalar1=S32)
                            nc.sync.dma_start(
                                out=vda[tcg * 4 + tt, :,
                                        nch * 8:(nch + 1) * 8, :],
                                in_=vt)
                    if tcg < 2:
                        # Q projection for own tokens, inside the loop so PE
                        # work covers the next chunk's stats chain
                        for m in range(NCC):
                            qp = ps_rot()
                            for p in range(4):
                                nc.tensor.matmul(qp, wtq[p][:, :, ts(m, 128)],
                                                 h_t[:, 2 * p:2 * p + 2, :],
                                                 start=(p == 0), stop=(p == 3),
                                                 perf_mode=DR,
                                                 skip_group_check=True)
                            nc.scalar.activation(
                                out=qz0[0:64, m, ts(tcg, 512)], in_=qp[0:64, :],
                                func=AF.Identity, bias=bq_sb[0:64, m:m + 1],
                                scale=S32)
                            nc.scalar.activation(
                                out=qz1[64:128, m, ts(tcg, 512)],
                                in_=qp[64:128, :],
                                func=AF.Identity, bias=bq_sb[64:128, m:m + 1],
                                scale=S32)

            # ---- attention + proj + MLP, phase-interleaved:
            #   B: attention qc0 (ACT exp-bound)
            #   C: attention qc1 with proj0/LN2(0)/MLP1(0) matmuls filling PE
            #   D: proj1 + MLP2(0) + LN2(1) + MLP(1)  (PE-dense)
            # x2 residual stays in SBUF as bf16 tiles (no DRAM round trip).
            with tc.tile_pool(name="vap", bufs=1) as vap, \
                 tc.tile_pool(name="expp", bufs=3) as expp, \
                 tc.tile_pool(name="stage", bufs=2) as stage, \
                 tc.tile_pool(name="x2p", bufs=16) as x2p, \
                 tc.tile_pool(name="mlp", bufs=1) as mlp, \
                 tc.tile_pool(name="h2p", bufs=2) as h2p, \
                 tc.tile_pool(name="outp", bufs=3) as outp:
                v_aug = vap.tile([128, 16, 16, 65], F8, name="v_aug")
                for ttg in range(16):
                    for nch in range(2):
                        nc.sync.dma_start(
                            out=v_aug[:, ttg, nch * 8:(nch + 1) * 8, :],
                            in_=vda[ttg, :, nch * 8:(nch + 1) * 8, :])

                def attn_head(qc, h, stp_ok, feeder=None):
                    hp = h // 2
                    p0 = (h % 2) * 64
                    qz = qz0 if h % 2 == 0 else qz1
                    avp = ps_mm()
                    for g in range(8):
                        scp = ps_stp() if (stp_ok and g % 3 == 2) else ps_sc2()
                        for j in range(2):
                            sk = g * 2 + j
                            nc.tensor.matmul(scp[:, j, :],
                                             k_sb[:, hp, ts(sk, 128)],
                                             qz[:, hp, ts(qc, 512)],
                                             start=True, stop=True)
                        et = expp.tile([128, 2, 512], F8, name="et", tag="et")
                        nc.scalar.activation(out=et, in_=scp, func=AF.Exp,
                                             scale=SCALE)
                        nc.tensor.matmul(avp[0:65, :],
                                         v_aug[:, 2 * g:2 * g + 2, h, :], et,
                                         start=(g == 0), stop=(g == 7),
                                         perf_mode=DR, skip_group_check=True)
                        if feeder is not None:
                            for _ in range(3):
                                next(feeder, None)
                    r_t = misc.tile([1, 512], F32, name="r_t", tag="r", bufs=2)
                    nc.vector.reciprocal(out=r_t, in_=avp[64:65, :])
                    rb_t = misc.tile([64, 512], F32, name="rb_t", tag="rb",
                                     bufs=2)
                    nc.gpsimd.partition_broadcast(rb_t, r_t)
                    nc.vector.scalar_tensor_tensor(
                        out=o_sb[p0:p0 + 64, hp, ts(qc, 512)],
                        in0=avp[0:64, :], scalar=32.0, in1=rb_t,
                        op0=MUL, op1=MUL)

                x2t = {0: [], 1: []}

                def proj_qc(qc, wtp):
                    st2 = ps_stp()
                    for m in range(NCC):
                        pp = ps_rot()
                        for p in range(4):
                            nc.tensor.matmul(pp, wtp[p][:, :, ts(m, 128)],
                                             o_sb[:, 2 * p:2 * p + 2,
                                                  ts(qc, 512)],
                                             start=(p == 0), stop=(p == 3),
                                             perf_mode=DR,
                                             skip_group_check=True)
                        xres = stage.tile([128, 512], F32, name="xres",
                                          tag="xres")
                        nc.sync.dma_start(out=xres, in_=xT[m, :, ts(qc, 512)])
                        xt = x2p.tile([128, 512], BF, name="x2t", tag="x2")
                        if has_bp:
                            ptmp = stage.tile([128, 512], F32, name="ptmp",
                                              tag="ptmp")
                            nc.scalar.activation(out=ptmp, in_=pp,
                                                 func=AF.Identity,
                                                 bias=bp_sb[:, m:m + 1],
                                                 scale=2.0 ** -10)
                            nc.vector.tensor_add(out=xt, in0=ptmp, in1=xres)
                        else:
                            nc.vector.scalar_tensor_tensor(
                                out=xt, in0=pp, scalar=2.0 ** -10,
                                in1=xres, op0=MUL, op1=ADD)
                        x2t[qc].append(xt)
                        xq2 = stage.tile([128, 512], BF, name="xq2",
                                         tag="xq2", bufs=3)
                        nc.vector.tensor_mul(out=xq2, in0=xt, in1=xt)
                        nc.tensor.matmul(st2[:, 0, :], ones_sc, xt,
                                         start=(m == 0), stop=(m == NCC - 1),
                                         skip_group_check=True)
                        nc.tensor.matmul(st2[:, 1, :], ones_sc, xq2,
                                         start=(m == 0), stop=(m == NCC - 1),
                                         skip_group_check=True)
                    return stats_finish(st2, "mstat", bufs=6)

                def ln2_apply(qc, mu2, s2):
                    h2 = h2p.tile([128, 8, 512], BF, name="h2", tag="h2")
                    for c in range(NCC):
                        d2 = stage.tile([128, 512], F32, name="d2", tag="d2",
                                        bufs=4)
                        nc.vector.tensor_sub(out=d2, in0=x2t[qc][c], in1=mu2)
                        nc.vector.tensor_mul(out=h2[:, c, :], in0=d2, in1=s2)
                    return h2

                def mlp1_d(h2, z_sb):
                    for mg in range(8):
                        if mg % 2 == 0:
                            t_a, t_b = ps_sc2(), ps_sc2()
                            accs = [t_a[:, 0, :], t_a[:, 1, :],
                                    t_b[:, 0, :], t_b[:, 1, :]]
                        else:
                            t_a = ps_stp()
                            accs = [t_a[:, 0, :], t_a[:, 1, :],
                                    ps_mm(), ps_mm()]
                        w1t = [wpool.tile([128, 512], BF, name="w1t",
                                          tag="w1s", bufs=12)
                               for _ in range(NCC)]
                        for k in range(NCC):
                            nc.sync.dma_start(out=w1t[k], in_=w1[mg, k])
                        for k in range(NCC):
                            for m4 in range(4):
                                nc.tensor.matmul(accs[m4],
                                                 w1t[k][:, ts(m4, 128)],
                                                 h2[:, k, :], start=(k == 0),
                                                 stop=(k == NCC - 1),
                                                 skip_group_check=True)
                        for m4 in range(4):
                            m = mg * 4 + m4
                            nc.vector.tensor_scalar(
                                out=z_sb[:, m, :], in0=accs[m4],
                                scalar1=b1_sb[:, m:m + 1], scalar2=0.0,
                                op0=ADD, op1=MAX)

                def mlp2(qc, z_sb):
                    accs = acc8()
                    for k in range(NFC):
                        w2t = wpool.tile([128, 1024], BF, name="w2t",
                                         tag="w2s", bufs=3)
                        nc.sync.dma_start(out=w2t, in_=w2[k])
                        for m in range(8):
                            nc.tensor.matmul(accs[m], w2t[:, ts(m, 128)],
                                             z_sb[:, k, :], start=(k == 0),
                                             stop=(k == NFC - 1),
                                             skip_group_check=True)
                    for m in range(8):
                        ot = outp.tile([128, 512], F32, name="ot", tag="ot")
                        nc.vector.scalar_tensor_tensor(
                            out=ot, in0=accs[m], scalar=b2_sb[:, m:m + 1],
                            in1=x2t[qc][m], op0=ADD, op1=ADD)
                        nc.sync.dma_start(out=outT[m, :, ts(qc, 512)], in_=ot)

                def mlp1_feed(h2, z_sb):
                    """Yields once per emitted PE matmul so the attention
                    loop can pace MLP1 work between score/AV groups."""
                    for mg in range(8):
                        w1t = [wpool.tile([128, 512], BF, name="w1t",
                                          tag="w1s", bufs=12)
                               for _ in range(NCC)]
                        for k in range(NCC):
                            nc.sync.dma_start(out=w1t[k], in_=w1[mg, k])
                        for half in range(2):
                            acc = ps_stp()
                            for k in range(NCC):
                                for m2 in range(2):
                                    nc.tensor.matmul(
                                        acc[:, m2, :],
                                        w1t[k][:, ts(half * 2 + m2, 128)],
                                        h2[:, k, :], start=(k == 0),
                                        stop=(k == NCC - 1),
                                        skip_group_check=True)
                                    yield
                            for m2 in range(2):
                                m = mg * 4 + half * 2 + m2
                                nc.vector.tensor_scalar(
                                    out=z_sb[:, m, :], in0=acc[:, m2, :],
                                    scalar1=b1_sb[:, m:m + 1], scalar2=0.0,
                                    op0=ADD, op1=MAX)

                cstate = {}

                def phasec_work(wtp0):
                    yield  # first pull is a no-op so head 0 starts clean
                    mu20, s20 = proj_qc(0, wtp0)
                    yield
                    cstate["h2_0"] = ln2_apply(0, mu20, s20)
                    cstate["z_0"] = mlp.tile([128, 32, 512], BF, name="z_sb",
                                             tag="z")
                    yield
                    yield from mlp1_feed(cstate["h2_0"], cstate["z_0"])

                wtp0 = [wpool.tile([128, 2, 1024], F8, name="wtp", tag="w")
                        for _ in range(4)]
                for p in range(4):
                    nc.sync.dma_start(out=wtp0[p], in_=wproj[p])
                # phase B: attention qc0 (ACT exp-bound; PE has slack)
                for h in range(H):
                    attn_head(0, h, stp_ok=True)
                wtp1 = [wpool.tile([128, 2, 1024], F8, name="wtp", tag="w")
                        for _ in range(4)]
                for p in range(4):
                    nc.sync.dma_start(out=wtp1[p], in_=wproj[p])
                # phase C: attention qc1 with proj0 + LN2(0) + MLP1(0) fed in
                # a few matmuls per score group so the exp stream never gaps
                feeder = phasec_work(wtp0)
                next(feeder)
                for h in range(H):
                    attn_head(1, h, stp_ok=False,
                              feeder=feeder if h >= 1 else None)
                for _ in feeder:
                    pass
                h2_0, z_0 = cstate["h2_0"], cstate["z_0"]
                # phase D: PE-dense tail
                mu21, s21 = proj_qc(1, wtp1)
                mlp2(0, z_0)
                h2_1 = ln2_apply(1, mu21, s21)
                z_1 = mlp.tile([128, 32, 512], BF, name="z_sb", tag="z")
                mlp1_d(h2_1, z_1)
                mlp2(1, z_1)


def _build(has_bv, has_bp):
    from concourse import bacc, mybir, tile
    F32 = mybir.dt.float32
    BF = mybir.dt.bfloat16
    F8 = mybir.dt.float8e4

    nc = bacc.Bacc("TRN2", target_bir_lowering=False, debug=False,
                   enable_asserts=False, num_devices=8)
    aps = {}
    aps["xT"] = nc.dram_tensor("xT", [8, 128, T], F32, kind="ExternalInput").ap()
    for n in ("wq", "wk", "wv", "wproj"):
        aps[n] = nc.dram_tensor(n, [4, 128, 2, 1024], F8,
                                kind="ExternalInput").ap()
    aps["w1"] = nc.dram_tensor("w1", [8, 8, 128, 512], BF, kind="ExternalInput").ap()
    aps["w2"] = nc.dram_tensor("w2", [32, 128, 1024], BF, kind="ExternalInput").ap()
    for n in ("bq", "bk", "bp", "b2c"):
        aps[n] = nc.dram_tensor(n, [128, 8], F32, kind="ExternalInput").ap()
    aps["b1c"] = nc.dram_tensor("b1c", [128, 32], F32, kind="ExternalInput").ap()
    if has_bv:
        aps["bvrow"] = nc.dram_tensor("bvrow", [1, 1024], F32,
                                      kind="ExternalInput").ap()
    aps["x2d"] = nc.dram_tensor("x2d", [8, 128, TO], F32).ap()
    aps["vda"] = nc.dram_tensor("vda", [16, 128, 16, 65], F8).ap()
    aps["outT"] = nc.dram_tensor("outT", [8, 128, TO], F32,
                                 kind="ExternalOutput").ap()

    with tile.TileContext(nc) as tcx:
        _emit(nc, tcx, aps, has_bv, has_bp)
    nc.compile()
    return nc


def _prep_inputs(x, Wq, Wk, Wv, Wproj, bproj, W1, b1, W2, b2, g1, be1, g2, be2):
    """Host-side prep: fold LN affine into weights/biases, cast, lay out."""
    x = np.asarray(x, np.float32)
    g1 = np.asarray(g1, np.float32)
    be1 = np.asarray(be1, np.float32)
    g2 = np.asarray(g2, np.float32)
    be2 = np.asarray(be2, np.float32)

    def to2d(w):  # (H, C, hd) -> (C, H*hd)
        return np.asarray(w, np.float32).transpose(1, 0, 2).reshape(C, C)

    wq2, wk2, wv2 = to2d(Wq), to2d(Wk), to2d(Wv)
    Wproj = np.asarray(Wproj, np.float32)
    W1 = np.asarray(W1, np.float32)
    W2 = np.asarray(W2, np.float32)

    wq_e, wk_e, wv_e = g1[:, None] * wq2, g1[:, None] * wk2, g1[:, None] * wv2
    w1_e = g2[:, None] * W1
    bias_q = be1 @ wq2
    bias_k = be1 @ wk2
    bias_v = be1 @ wv2
    bias_1 = np.asarray(b1, np.float32) + be2 @ W1

    def wpair8(w):
        """(C, N) f32, scaled x32 -> [4, 128, 2, N] fp8e4m3 DoubleRow pairs:
        pair p slot j holds contraction rows 256p+128j .. +127."""
        w8 = np.clip(w * 32.0, -240.0, 240.0).astype(F8NP)
        return np.ascontiguousarray(
            w8.reshape(4, 2, 128, -1).transpose(0, 2, 1, 3))

    def bvec(v):  # (N,) -> (128, N//128) partition-major
        return np.ascontiguousarray(np.asarray(v, np.float32).reshape(-1, 128).T)

    shared = {
        "wq": wpair8(wq_e), "wk": wpair8(wk_e), "wv": wpair8(wv_e),
        "wproj": wpair8(Wproj),
        "w1": np.ascontiguousarray(
            w1_e.reshape(NCC, 128, 8, 512).transpose(2, 0, 1, 3).astype(BF16)),
        "w2": np.ascontiguousarray(W2.reshape(NFC, 128, C).astype(BF16)),
        "bq": bvec(bias_q), "bk": bvec(bias_k),
        "bp": bvec(np.asarray(bproj, np.float32)),
        "b2c": bvec(np.asarray(b2, np.float32)), "b1c": bvec(bias_1),
    }
    has_bv = bool(np.any(bias_v != 0.0))
    has_bp = bool(np.any(np.asarray(bproj, np.float32) != 0.0))
    if has_bv:
        shared["bvrow"] = np.ascontiguousarray(bias_v.reshape(1, C))

    in_maps = []
    for core in range(8):
        b, half = core // 2, core % 2
        xt = x[b].T  # (C, T)
        own = xt[:, half * TO:(half + 1) * TO]
        oth = xt[:, (1 - half) * TO:(2 - half) * TO]
        m = dict(shared)
        m["xT"] = np.ascontiguousarray(
            np.concatenate([own, oth], axis=1).reshape(NCC, 128, T))
        in_maps.append(m)
    return in_maps, has_bv, has_bp


def kernel(x, Wq, Wk, Wv, Wproj, bproj, W1, b1, W2, b2, g1, be1, g2, be2):
    global _BUILT
    from concourse.bass_utils import run_bass_kernel_spmd

    in_maps, has_bv, has_bp = _prep_inputs(x, Wq, Wk, Wv, Wproj, bproj, W1, b1,
                                           W2, b2, g1, be1, g2, be2)
    if _BUILT is None or _BUILT[1] != (has_bv, has_bp):
        _BUILT = (_build(has_bv, has_bp), (has_bv, has_bp))
    nc = _BUILT[0]
    res = run_bass_kernel_spmd(nc, in_maps, core_ids=list(range(8)))
    out = np.empty((B, T, C), np.float32)
    for core in range(8):
        b, half = core // 2, core % 2
        o = res.results[core]["outT"].reshape(C, TO)  # (feature, token)
        out[b, half * TO:(half + 1) * TO, :] = o.T
    return out

